# revision 9
# baseline (speedup 1.0000x reference)
"""BBoxHead kernel for 8 Trainium2 NeuronCores.

Reference computation (per roi):
  x1 = relu(bn1(pooled_rois . conv1_w + b1))      # full 7x7x256 contraction -> 1024
  x2 = relu(bn2(x1 @ conv2_w + b2))               # 1024 -> 1024
  logits = x2 @ logits_w + logits_b               # 1024 -> 81
  probs  = softmax(logits)
  deltas = x2 @ delta_w + delta_b                 # 1024 -> 324 -> [81, 4]

Sharding: data-parallel over the 2000-roi axis (250 rois/core, padded to 256);
weights replicated. Activations are kept feature-major on-chip (X^T layout,
[features, rois]) so every matmul consumes operands K-on-partitions with zero
on-device transposes of activations; the host pre-transposes the pooled rois
once. BN is folded into a per-feature affine on the host and applied fused
with ReLU in a single ScalarE activation per tile (PSUM -> SBUF).
"""

import os
import sys

sys.path.insert(0, "/opt/trn_rl_repo")
import numpy as np

N_ROIS = 2000
K1 = 12544          # 7*7*256 contraction for conv1
HID = 1024
NCLS = 81
NCLS_P = 128        # logits head padded to a full PE tile
NDEL = 324
NDEL_P = 384        # delta head padded to 3 full PE tiles
P = 128
KT1 = K1 // P       # 98 contraction tiles for conv1
FT = HID // P       # 8 feature tiles
NCORES = 8
RPC = N_ROIS // NCORES  # 250 rois per core
NR = 256            # padded rois per core (f32r needs free dim >= 256)
BN_EPS = 1e-3

# f32r ("relaxed" fp32) runs the PE at 4x the fp32 rate for free dim >= 256.
USE_F32R = os.environ.get("BBOX_MM_DTYPE", "f32r") == "f32r"

_CACHE: dict = {}


def _build_program():
    from concourse import bacc
    import concourse.mybir as mybir
    import concourse.tile as tile
    from concourse.masks import make_identity

    f32 = mybir.dt.float32
    mm_dt = mybir.dt.float32r if USE_F32R else f32
    AF = mybir.ActivationFunctionType
    AX = mybir.AxisListType

    nc = bacc.Bacc("TRN2", target_bir_lowering=False, debug=False,
                   num_devices=NCORES)

    a_t = nc.dram_tensor("a_t", [K1, NR], f32, kind="ExternalInput")
    w1 = nc.dram_tensor("w1", [K1, HID], f32, kind="ExternalInput")
    w2 = nc.dram_tensor("w2", [HID, HID], f32, kind="ExternalInput")
    w3 = nc.dram_tensor("w3", [HID, NCLS_P], f32, kind="ExternalInput")
    w4 = nc.dram_tensor("w4", [HID, NDEL_P], f32, kind="ExternalInput")
    s1 = nc.dram_tensor("s1", [HID], f32, kind="ExternalInput")
    t1 = nc.dram_tensor("t1", [HID], f32, kind="ExternalInput")
    s2 = nc.dram_tensor("s2", [HID], f32, kind="ExternalInput")
    t2 = nc.dram_tensor("t2", [HID], f32, kind="ExternalInput")
    b3 = nc.dram_tensor("b3", [NCLS_P], f32, kind="ExternalInput")
    b4 = nc.dram_tensor("b4", [NDEL_P], f32, kind="ExternalInput")
    logits_out = nc.dram_tensor("logits_out", [NR, NCLS], f32, kind="ExternalOutput")
    probs_out = nc.dram_tensor("probs_out", [NR, NCLS], f32, kind="ExternalOutput")
    deltas_out = nc.dram_tensor("deltas_out", [NR, NDEL], f32, kind="ExternalOutput")

    AG, KPG = 14, 7  # stream a_t in 14 groups of 7 k-tiles

    with tile.TileContext(nc) as tc:
        with (
            tc.tile_pool(name="singles", bufs=1) as singles,
            tc.tile_pool(name="astream", bufs=4) as apool,
            tc.tile_pool(name="wstream", bufs=4) as wpool,
            tc.tile_pool(name="psum", bufs=1, space="PSUM") as psum,
            tc.tile_pool(name="small", bufs=4) as small,
        ):
            ident = singles.tile([P, P], f32)
            make_identity(nc, ident)

            def vec_tile(dram, cols):
                t = singles.tile([P, cols], f32, tag=dram.name + "_sb")
                nc.sync.dma_start(t, dram.ap().rearrange("(o p) -> p o", p=P))
                return t

            s1_sb = vec_tile(s1, FT)
            t1_sb = vec_tile(t1, FT)
            s2_sb = vec_tile(s2, FT)
            t2_sb = vec_tile(t2, FT)
            b3_sb = vec_tile(b3, 1)
            b4_sb = vec_tile(b4, 3)

            w2_sb = singles.tile([P, FT, HID], mm_dt)
            nc.sync.dma_start(w2_sb, w2.ap().rearrange("(kt p) f -> p kt f", p=P).bitcast(mm_dt))
            w3_sb = singles.tile([P, FT, NCLS_P], mm_dt)
            nc.sync.dma_start(w3_sb, w3.ap().rearrange("(kt p) f -> p kt f", p=P).bitcast(mm_dt))
            w4_sb = singles.tile([P, FT, NDEL_P], mm_dt)
            nc.sync.dma_start(w4_sb, w4.ap().rearrange("(kt p) f -> p kt f", p=P).bitcast(mm_dt))

            # conv1: accumulate X1^T = W1^T @ A^T over 98 k-tiles.
            # One PSUM bank per feature tile ([*, f, :256] of an 8-bank tile).
            a_t3 = a_t.ap().rearrange("(kt p) n -> p kt n", p=P)
            c1 = psum.tile([P, FT, 512], f32, tag="acc")
            a_g = None
            for k in range(KT1):
                if k % KPG == 0:
                    a_g = apool.tile([P, KPG, NR], mm_dt, tag="a")
                    g = k // KPG
                    nc.sync.dma_start(
                        a_g, a_t3[:, g * KPG:(g + 1) * KPG, :].bitcast(mm_dt))
                w1_k = wpool.tile([P, HID], mm_dt, tag="w1")
                nc.sync.dma_start(w1_k, w1.ap()[k * P:(k + 1) * P, :].bitcast(mm_dt))
                rhs = a_g[:, k % KPG, :]
                for f in range(FT):
                    nc.tensor.matmul(
                        c1[:, f, :NR],
                        lhsT=w1_k[:, f * P:(f + 1) * P],
                        rhs=rhs,
                        start=(k == 0),
                        stop=(k == KT1 - 1),
                    )

            # BN1 + ReLU fused: x1 = relu(c1 * s1 + t1), PSUM -> SBUF
            x1_sb = singles.tile([P, FT, NR], mm_dt)
            for f in range(FT):
                nc.scalar.activation(
                    out=x1_sb[:, f, :], in_=c1[:, f, :NR], func=AF.Relu,
                    bias=t1_sb[:, f:f + 1], scale=s1_sb[:, f:f + 1],
                )

            # conv2: X2^T = W2^T @ X1^T
            c2 = psum.tile([P, FT, 512], f32, tag="acc")
            for f in range(FT):
                for k in range(FT):
                    nc.tensor.matmul(
                        c2[:, f, :NR],
                        lhsT=w2_sb[:, k, f * P:(f + 1) * P],
                        rhs=x1_sb[:, k, :],
                        start=(k == 0),
                        stop=(k == FT - 1),
                    )

            x2_sb = singles.tile([P, FT, NR], mm_dt)
            for f in range(FT):
                nc.scalar.activation(
                    out=x2_sb[:, f, :], in_=c2[:, f, :NR], func=AF.Relu,
                    bias=t2_sb[:, f:f + 1], scale=s2_sb[:, f:f + 1],
                )

            # heads: logits^T into bank 0, deltas^T into banks 1..3
            c3 = psum.tile([P, FT, 512], f32, tag="acc")
            for k in range(FT):
                nc.tensor.matmul(
                    c3[:, 0, :NR],
                    lhsT=w3_sb[:, k, :],
                    rhs=x2_sb[:, k, :],
                    start=(k == 0), stop=(k == FT - 1),
                )
            for m in range(3):
                for k in range(FT):
                    nc.tensor.matmul(
                        c3[:, 1 + m, :NR],
                        lhsT=w4_sb[:, k, m * P:(m + 1) * P],
                        rhs=x2_sb[:, k, :],
                        start=(k == 0), stop=(k == FT - 1),
                    )

            l_sb = small.tile([P, NR], f32, tag="l")
            nc.scalar.activation(out=l_sb, in_=c3[:, 0, :NR], func=AF.Identity,
                                 bias=b3_sb[:, 0:1], scale=1.0)
            d_sb = small.tile([P, 3, NR], f32, tag="d")
            for m in range(3):
                nc.scalar.activation(out=d_sb[:, m, :], in_=c3[:, 1 + m, :NR],
                                     func=AF.Identity, bias=b4_sb[:, m:m + 1], scale=1.0)

            # transpose heads back to roi-major: 2 logit blocks + 6 delta blocks
            c4 = psum.tile([P, FT, 512], f32, tag="acc")
            for j in range(2):
                nc.tensor.transpose(c4[:, j, :P], l_sb[:, j * P:(j + 1) * P], ident)
            for m in range(3):
                for j in range(2):
                    nc.tensor.transpose(c4[:, 2 + m * 2 + j, :P],
                                        d_sb[:, m, j * P:(j + 1) * P], ident)

            lg_sb = small.tile([P, 2, NCLS], f32, tag="lg")
            pr_sb = small.tile([P, 2, NCLS], f32, tag="pr")
            dl_sb = small.tile([P, 2, NDEL], f32, tag="dl")
            for j in range(2):
                nc.vector.tensor_copy(lg_sb[:, j, :], c4[:, j, :NCLS])
                negmax = small.tile([P, 1], f32, tag="nm")
                nc.vector.reduce_max(negmax, c4[:, j, :NCLS], axis=AX.X, negate=True)
                esum = small.tile([P, 1], f32, tag="es")
                nc.scalar.activation(out=pr_sb[:, j, :], in_=c4[:, j, :NCLS],
                                     func=AF.Exp, bias=negmax, scale=1.0,
                                     accum_out=esum)
                rsum = small.tile([P, 1], f32, tag="rs")
                nc.vector.reciprocal(rsum, esum)
                nc.vector.tensor_scalar_mul(pr_sb[:, j, :], pr_sb[:, j, :], rsum)
                for m in range(3):
                    mw = NDEL - m * P if m == 2 else P
                    nc.vector.tensor_copy(dl_sb[:, j, m * P:m * P + mw],
                                          c4[:, 2 + m * 2 + j, :mw])

            nc.sync.dma_start(logits_out.ap().rearrange("(j p) c -> p j c", p=P), lg_sb)
            nc.sync.dma_start(probs_out.ap().rearrange("(j p) c -> p j c", p=P), pr_sb)
            nc.sync.dma_start(deltas_out.ap().rearrange("(j p) c -> p j c", p=P), dl_sb)

    nc.compile()
    return nc


def get_program():
    if "nc" not in _CACHE:
        _CACHE["nc"] = _build_program()
    return _CACHE["nc"]


def _round_f32r(x):
    """Round fp32 to the PE's FP32r (11-bit mantissa, TF32-like) format so the
    on-device rounding step is a no-op and accuracy matches round-to-nearest."""
    x = np.ascontiguousarray(x, np.float32)
    b = x.view(np.uint32).astype(np.uint64)
    return (((b + 0x800) & 0xFFFFF000).astype(np.uint32)).view(np.float32)


def prepare_in_maps(pooled_rois, conv1_w, conv1_b, bn1_gamma, bn1_beta, bn1_mean,
                    bn1_var, conv2_w, conv2_b, bn2_gamma, bn2_beta, bn2_mean,
                    bn2_var, logits_w, logits_b, delta_w, delta_b):
    f = np.float32
    a_all = np.asarray(pooled_rois, f).reshape(N_ROIS, K1).T  # [K1, N_ROIS]

    def fold(gamma, beta, mean, var, conv_b):
        s = np.asarray(gamma, np.float64) / np.sqrt(np.asarray(var, np.float64) + BN_EPS)
        t = (np.asarray(conv_b, np.float64) - np.asarray(mean, np.float64)) * s \
            + np.asarray(beta, np.float64)
        return s.astype(f), t.astype(f)

    s1_np, t1_np = fold(bn1_gamma, bn1_beta, bn1_mean, bn1_var, conv1_b)
    s2_np, t2_np = fold(bn2_gamma, bn2_beta, bn2_mean, bn2_var, conv2_b)

    w3_np = np.zeros((HID, NCLS_P), f)
    w3_np[:, :NCLS] = np.asarray(logits_w, f)
    b3_np = np.zeros(NCLS_P, f)
    b3_np[:NCLS] = np.asarray(logits_b, f)
    w4_np = np.zeros((HID, NDEL_P), f)
    w4_np[:, :NDEL] = np.asarray(delta_w, f)
    b4_np = np.zeros(NDEL_P, f)
    b4_np[:NDEL] = np.asarray(delta_b, f)

    w1_np = np.ascontiguousarray(np.asarray(conv1_w, f).reshape(K1, HID))
    w2_np = np.ascontiguousarray(np.asarray(conv2_w, f))
    if USE_F32R:
        w1_np = _round_f32r(w1_np)
        w2_np = _round_f32r(w2_np)
        w3_np = _round_f32r(w3_np)
        w4_np = _round_f32r(w4_np)
        a_all = _round_f32r(a_all)

    shared = {
        "w1": w1_np, "w2": w2_np, "w3": w3_np, "w4": w4_np,
        "s1": s1_np, "t1": t1_np, "s2": s2_np, "t2": t2_np,
        "b3": b3_np, "b4": b4_np,
    }
    in_maps = []
    for c in range(NCORES):
        a_c = np.zeros((K1, NR), f)
        a_c[:, :RPC] = a_all[:, c * RPC:(c + 1) * RPC]
        in_maps.append({"a_t": a_c, **shared})
    return in_maps


def gather_outputs(results):
    logits = np.concatenate([r["logits_out"][:RPC] for r in results], axis=0)
    probs = np.concatenate([r["probs_out"][:RPC] for r in results], axis=0)
    deltas = np.concatenate([r["deltas_out"][:RPC] for r in results], axis=0)
    return logits, probs, deltas.reshape(N_ROIS, NCLS, 4)


def kernel(**inputs):
    from concourse.bass_utils import run_bass_kernel_spmd

    nc = get_program()
    in_maps = prepare_in_maps(**inputs)
    trace = bool(os.environ.get("BBOX_TRACE"))
    kwargs = {}
    if trace:
        kwargs = {"trace": True, "tmpdir": os.environ.get("BBOX_TRACE_DIR") or None}
    res = run_bass_kernel_spmd(nc, in_maps, core_ids=list(range(NCORES)), **kwargs)
    if trace:
        print(f"HW exec time: {res.exec_time_ns} ns")
        if res.instructions_and_trace:
            print("trace path:", res.instructions_and_trace[1])
        _CACHE["last_results"] = res
    return gather_outputs(res.results)
